# revision 26
# baseline (speedup 1.0000x reference)
"""DLRM dot-interaction kernel for Trainium2 (8 NeuronCores, batch-sharded).

Per sample b: T = concat(dense[b], embs[b]) -> [27, 128]; Z = T @ T^T;
output = strict upper triangle of Z -> [351] fp32 (computed fp16 on-chip,
upcast on host; fp16 quantization of outputs adds ~2.5e-4 rel err).

Per-core plan (2048 samples, 16 blocks of 128):
  - SWDGE cast-DMA loads (fp32 -> fp16), b-major; block 0 in two half
    tiles so the PE starts transposing ~halfway through its load, later
    blocks one tile each, prefetched ~3 ahead (measured: finer chunking
    or HWDGE fp32 for block 0 only slows first-data arrival -- the SDMA
    round-robins across queued transfers).
  - PE transposes each [128 b, 128 d] feature slab (transpose-mode fp16,
    ~107ns, LDW-paced at the fixed 1.2 GHz NX column rate).  Four tp
    groups per block; each group's PSUM->SBUF copy is split DVE-front/
    ACT-back so the tp slot drains fast.
  - Per-sample gram matmuls: lhsT = [128 d, 26] (features 0..25), rhs =
    [128 d, 26] (features 1..26) strided slices of f-major Tt -- the
    strict upper triangle only needs rows m<26 and cols n>0.  out -> PSUM
    zp[32*g + m, q*32 + (n-1)] fp32 (sample s = q*4 + g), ~33ns/sample
    (serial LDW+MM; ldw-opt and FWL are unavailable in this toolchain --
    measured).
  - One DVE StreamTranspose per block swaps m<->q inside each quadrant:
    PSUM -> SBUF Zb fp32 slot t of a 4-block pack group.  Triu pack: 26
    strided-run copies per group (alternating DVE/ACT) cast fp32 -> fp16
    into Pk; HWDGE DMAs with 702B runs write out[b, :] fp16 (partition
    (g,q) -> row q*4+g).
  - The last block packs alone with a DVE-heavy copy split, and zb/pk
    pools are triple-buffered (no WAR wait on the previous pack group),
    so the post-PE drain tail after the final matmul is short.
"""

import numpy as np

B, NUM_EMBS, D = 16384, 26, 128
N_CORES = 8
BC = B // N_CORES  # 2048 samples per core
BLK = 128          # samples per block
NF = NUM_EMBS + 1  # 27 features
FP = 32            # feature pitch in the Z PSUM tile
NPAIR = NF * (NF - 1) // 2  # 351
GROUPS = (7, 7, 7, 6)    # transpose groups per block
NPK = 4                  # blocks per pack group (except the tail)

_CACHE = {}


def build(bc=BC):
    import concourse.bacc as bacc
    import concourse.mybir as mybir
    from concourse.tile import TileContext
    from concourse.masks import make_identity

    fp16 = mybir.dt.float16
    fp32 = mybir.dt.float32

    nc = bacc.Bacc("TRN2", target_bir_lowering=False, debug=False)
    dense_t = nc.dram_tensor("dense", (bc, D), fp32, kind="ExternalInput")
    embs_t = nc.dram_tensor("embs", (bc, NUM_EMBS, D), fp32, kind="ExternalInput")
    out_t = nc.dram_tensor("out", (bc, NPAIR), fp16, kind="ExternalOutput")

    nblk = bc // BLK
    # pack groups of NPK blocks; the last block packs alone so the final
    # drain chain (ST + pack + DMA) after the last matmul is short
    pgroups = []
    b = 0
    while b < nblk - 1:
        hi = min(b + NPK, nblk - 1)
        pgroups.append(tuple(range(b, hi)))
        b = hi
    pgroups.append((nblk - 1,))
    pg_of = {}
    for grp in pgroups:
        for blk in grp:
            pg_of[blk] = grp

    with TileContext(nc) as tc:
        with (
            tc.tile_pool(name="consts", bufs=1) as consts,
            tc.tile_pool(name="xin", bufs=1) as xpool,
            tc.tile_pool(name="tt", bufs=5) as ttpool,
            tc.tile_pool(name="zb", bufs=3) as zbpool,
            tc.tile_pool(name="pk", bufs=3) as pkpool,
            tc.tile_pool(name="tp", bufs=4, space="PSUM") as tppool,
            tc.tile_pool(name="zp", bufs=2, space="PSUM") as zppool,
        ):
            ident = consts.tile([128, 128], fp16)
            make_identity(nc, ident)

            dview = dense_t.ap()  # [bc, 128]
            eview = embs_t.ap().rearrange("b f d -> b (f d)")  # [bc, 3328]
            oview = out_t.ap()  # [bc, 351]

            xmap = {}   # blk -> list of (tile, f0, nf) segments
            tts = {}
            zbs = {}    # grp -> (zb, Pk)

            def emit_load(blk):
                b0 = blk * BLK
                if blk <= 2:
                    # two half-tiles so transposes can start after ~half
                    # the load lands -- covers the pipeline-fill phase
                    Xa = xpool.tile([BLK, 14 * D], fp16, tag="Xh0", bufs=3, name="Xa")
                    nc.gpsimd.dma_start(out=Xa[:, 0:D], in_=dview[b0 : b0 + BLK])
                    nc.gpsimd.dma_start(
                        out=Xa[:, D:], in_=eview[b0 : b0 + BLK, : 13 * D]
                    )
                    Xb = xpool.tile([BLK, 13 * D], fp16, tag="Xh1", bufs=3, name="Xb")
                    nc.gpsimd.dma_start(out=Xb[:, :], in_=eview[b0 : b0 + BLK, 13 * D :])
                    xmap[blk] = [(Xa, 0, 14), (Xb, 14, 13)]
                else:
                    X = xpool.tile([BLK, NF * D], fp16, tag="X", bufs=6, name="X")
                    nc.gpsimd.dma_start(out=X[:, 0:D], in_=dview[b0 : b0 + BLK])
                    nc.gpsimd.dma_start(out=X[:, D:], in_=eview[b0 : b0 + BLK])
                    xmap[blk] = [(X, 0, NF)]

            def _slab(blk, f):
                for tile, f0, nf in xmap[blk]:
                    if f0 <= f < f0 + nf:
                        c0 = (f - f0) * D
                        return tile[:, c0 : c0 + D]
                raise AssertionError

            def emit_transpose_group(blk, ci):
                """One tp group: slab transposes + split PSUM->SBUF copy.

                The copy is split DVE-front/ACT-back so the tp slot frees
                quickly even when one queue is busy.  Block 0 runs fp32
                (HWDGE-loaded, uncast); its copies cast fp32 -> fp16.
                """
                groups = GROUPS
                if ci == 0:
                    tts[blk] = ttpool.tile([128, NF * D], fp16, tag="Tt", name="Tt")
                Tt = tts[blk]
                c0 = sum(groups[:ci])
                cf = groups[ci]
                tp = tppool.tile([128, 7 * BLK], fp16, tag="tp", name="tp")
                for j in range(cf):
                    nc.tensor.transpose(
                        tp[:, j * BLK : (j + 1) * BLK], _slab(blk, c0 + j), ident
                    )
                h = (cf * BLK) // 2
                nc.vector.tensor_copy(
                    out=Tt[:, c0 * BLK : c0 * BLK + h], in_=tp[:, :h]
                )
                nc.scalar.copy(
                    Tt[:, c0 * BLK + h : (c0 + cf) * BLK], tp[:, h : cf * BLK]
                )
                if ci == len(groups) - 1:
                    del xmap[blk]

            def emit_st(zp, zb, t, np_):
                """StreamTranspose PSUM zp -> Zb slot t (np_ partitions)."""
                inv = zp.rearrange("p (q n) -> p n q", n=FP)[0 : np_, 0 : NF - 1, :]
                outv = zb.rearrange("p (t m n) -> p t n m", t=NPK, n=FP)[
                    0 : np_, t, 0 : NF - 1, :
                ]
                nc.vector.transpose(out=outv, in_=inv)

            def emit_pack(zb, Pk, nt, np_, b0, gn, tail=False):
                """Triu pack fp32->fp16 + out DMA for nt block-slots.

                tail=True puts most copies on DVE (low per-op overhead) so
                the final drain chain is short; otherwise alternate."""
                zbv = zb.rearrange("p (t m n) -> p t m n", t=NPK, n=FP)
                pkv = Pk.rearrange("p (t c) -> p t c", t=NPK)
                off = 0
                for m in range(NF - 1):
                    ln = NF - 1 - m
                    src = zbv[0:np_, 0:nt, m, m : m + ln]
                    dst = pkv[0:np_, 0:nt, off : off + ln]
                    on_act = (m < 5) if tail else (m % 2 == 1)
                    if on_act:
                        nc.scalar.copy(dst, src)
                    else:
                        nc.vector.tensor_copy(out=dst, in_=src)
                    off += ln
                if nt == 1:
                    # tail group: one 128-partition DMA (4 small 32-row
                    # DMAs are descriptor-dominated and ~3us slower)
                    ov = oview[b0 : b0 + gn * 32].rearrange(
                        "(q g) c -> g q c", g=gn
                    )
                    src = Pk[0:np_, 0:NPAIR].rearrange("(g q) c -> g q c", g=gn)
                    nc.sync.dma_start(out=ov, in_=src)
                    return
                ovq = oview[b0 : b0 + nt * gn * 32].rearrange(
                    "(t q g) c -> g q t c", t=nt, g=gn
                )
                pkg = Pk[0:np_].rearrange("(g q) (t c) -> g q t c", g=gn, t=NPK)[
                    :, :, 0:nt, :
                ]
                for g in range(gn):
                    nc.sync.dma_start(out=ovq[g], in_=pkg[g])

            gstate = {}  # blk -> (zp, Ttr)

            def emit_gram_quarter(blk, k):
                """Quarter (8 q-groups) of a block's gram matmuls.
                Emitted interleaved with the next block's transpose groups
                so the PE instruction stream consumption stays flat (dense
                gram bursts outrun the 16KB instruction-page prefetcher)."""
                if k == 0:
                    grp = pg_of[blk]
                    if grp.index(blk) == 0:
                        zb = zbpool.tile(
                            [128, NPK * FP * FP], fp32, tag="Zb", name="zb"
                        )
                        Pk = pkpool.tile([128, NPK * NPAIR], fp16, tag="Pk", name="Pk")
                        zbs[grp] = (zb, Pk)
                    Tt = tts.pop(blk)
                    Ttr = Tt.rearrange("d (f b) -> d b f", b=BLK)
                    zp = zppool.tile([128, FP * FP], fp32, tag="zp", name="zp")
                    gstate[blk] = (zp, Ttr)
                zp, Ttr = gstate[blk]
                for q in range(8 * k, 8 * k + 8):
                    for g in range(4):
                        s = q * 4 + g
                        nc.tensor.matmul(
                            zp[32 * g : 32 * g + NF - 1, q * FP : q * FP + NF - 1],
                            Ttr[:, s, 0 : NF - 1],
                            Ttr[:, s, 1:NF],
                            start=True,
                            stop=True,
                            tile_position=(0, 32 * g),
                        )
                if k == 3:
                    del gstate[blk]
                    grp = pg_of[blk]
                    zb, Pk = zbs[grp]
                    emit_st(zp, zb, grp.index(blk), 128)
                    if blk == grp[-1]:
                        del zbs[grp]
                        emit_pack(
                            zb,
                            Pk,
                            len(grp),
                            128,
                            grp[0] * BLK,
                            4,
                            tail=(blk == nblk - 1),
                        )

            def emit_gram_block(blk):
                for k in range(4):
                    emit_gram_quarter(blk, k)

            # Pipeline: loads prefetch ~3 blocks ahead of the transposes;
            # gram matmuls lag the transposes by one block.
            emit_load(0)
            emit_load(1)
            emit_load(2)
            emit_load(3)
            emit_load(4)
            for ci in range(4):
                emit_transpose_group(0, ci)
            for ci in range(4):
                emit_transpose_group(1, ci)
            emit_gram_block(0)
            emit_gram_block(1)
            for blk in range(2, nblk):
                if blk + 3 < nblk:
                    emit_load(blk + 3)
                for ci in range(4):
                    emit_transpose_group(blk, ci)
                    if blk >= 3:
                        emit_gram_quarter(blk - 1, ci)
            emit_gram_block(nblk - 1)

    nc.compile()
    return nc


def _get(bc=BC):
    if bc not in _CACHE:
        _CACHE[bc] = build(bc)
    return _CACHE[bc]


def kernel(dense: np.ndarray, embs: np.ndarray) -> np.ndarray:
    from concourse import bass_utils

    dense = np.ascontiguousarray(np.asarray(dense, dtype=np.float32))
    embs = np.ascontiguousarray(np.asarray(embs, dtype=np.float32))
    assert dense.shape == (B, D) and embs.shape == (B, NUM_EMBS, D)

    nc = _get()
    dsh = dense.reshape(N_CORES, BC, D)
    esh = embs.reshape(N_CORES, BC, NUM_EMBS, D)
    in_maps = [{"dense": dsh[i], "embs": esh[i]} for i in range(N_CORES)]
    res = bass_utils.run_bass_kernel_spmd(nc, in_maps, core_ids=list(range(N_CORES)))
    return np.concatenate([r["out"] for r in res.results], axis=0).astype(np.float32)
